# revision 1
# baseline (speedup 1.0000x reference)
"""GQA (32 q heads / 8 kv heads, head_dim 64, causal, QK-RMSNorm + RoPE) on 8 TRN2 cores.

Sharding: data-parallel over batch (2) x tensor-parallel over heads (4):
each core handles one batch element, 8 query heads, 2 kv heads, and produces
a partial output (its heads' slice of the Wo contraction); the host sums the
4 partials per batch element.

On-chip layout is "transposed": activations live as x^T / q^T / k^T with the
feature dim on partitions and tokens on the free dim, so Q/K/V/O projections
run with natural weight layouts and the softmax reduction (over keys) lands on
the PE via a ones-column appended to V (denominator accumulated in the same
matmul as the attention output).  All matmuls run in float32r.
"""

import numpy as np

import concourse.bass as bass
import concourse.mybir as mybir
import concourse.tile as tile
from concourse import bacc
from concourse.bass_utils import run_bass_kernel_spmd

# Problem config (hardcoded per contract)
B, T, D = 2, 2048, 2048
H, KV, HD = 32, 8, 64
GROUPS = H // KV
THETA = 10000.0
SCALE = 1.0 / np.sqrt(HD)
EPS = 1e-6

# Per-core sharding
HQL = H // 4          # 8 local q heads
KVL = KV // 4         # 2 local kv heads
FQ = HQL * HD         # 512
FKV = KVL * HD        # 128

# Tiling
P = 128
TB = 512              # token block
NTB = T // TB         # 4
NDC = D // P          # 16 contraction chunks
NKC = T // P          # 16 key chunks
NQC = FQ // P         # 4 q-proj chunks (2 heads each)

f32 = mybir.dt.float32
f32r = mybir.dt.float32r
AF = mybir.ActivationFunctionType
ALU = mybir.AluOpType


def _build_nc():
    nc = bacc.Bacc("TRN2", target_bir_lowering=False, debug=False, num_devices=8)

    eps_t = nc.alloc_sbuf_tensor("const-f32-eps", [128, 1], f32)
    nc.gpsimd.memset(eps_t.ap(), EPS)
    nc.const_aps.aps[(f32, EPS)] = eps_t.ap()
    nc.all_engine_barrier()

    xT_d = nc.dram_tensor("xT", [D, T], f32r, kind="ExternalInput")
    wq_d = nc.dram_tensor("wq", [D, FQ], f32r, kind="ExternalInput")
    wk_d = nc.dram_tensor("wk", [D, FKV], f32r, kind="ExternalInput")
    wv_d = nc.dram_tensor("wv", [D, FKV], f32r, kind="ExternalInput")
    wo_d = nc.dram_tensor("wo", [FQ, D], f32r, kind="ExternalInput")
    cosq_d = nc.dram_tensor("cosq", [P, T], f32, kind="ExternalInput")
    cosk_d = nc.dram_tensor("cosk", [P, T], f32, kind="ExternalInput")
    sin_d = nc.dram_tensor("sin", [P, T], f32, kind="ExternalInput")
    rqT_d = nc.dram_tensor("rqT", [P, P], f32r, kind="ExternalInput")
    rkT_d = nc.dram_tensor("rkT", [P, P], f32r, kind="ExternalInput")
    hsel_d = nc.dram_tensor("hsel", [P, 2], f32r, kind="ExternalInput")
    hexp_d = nc.dram_tensor("hexp", [2, P], f32r, kind="ExternalInput")
    e1_d = nc.dram_tensor("e1", [1, P], f32r, kind="ExternalInput")
    masks_d = nc.dram_tensor("masks", [P, 4, TB], f32r, kind="ExternalInput")
    ident_d = nc.dram_tensor("ident", [P, P], f32r, kind="ExternalInput")
    outT_d = nc.dram_tensor("outT", [D, T], f32, kind="ExternalOutput")

    with tile.TileContext(nc) as tc:
        with (
            tc.tile_pool(name="wpool", bufs=1) as wpool,
            tc.tile_pool(name="cpool", bufs=1) as cpool,
            tc.tile_pool(name="kvpool", bufs=1) as kvpool,
            tc.tile_pool(name="trig", bufs=1) as trig,
            tc.tile_pool(name="xpool", bufs=5) as xpool,
            tc.tile_pool(name="qpool", bufs=1) as qpool,
            tc.tile_pool(name="btmp", bufs=2) as btmp,
            tc.tile_pool(name="spool", bufs=2) as spool,
            tc.tile_pool(name="epool", bufs=6) as epool,
            tc.tile_pool(name="opool", bufs=1) as opool,
            tc.tile_pool(name="outp", bufs=2) as outp,
            tc.tile_pool(name="psum", bufs=6, space="PSUM") as psum,
            tc.tile_pool(name="psmall", bufs=2, space="PSUM") as psmall,
        ):
            # ---- persistent weights / constants ----
            wq_sb = wpool.tile([P, NDC, FQ], f32r)
            wk_sb = wpool.tile([P, NDC, FKV], f32r)
            wv_sb = wpool.tile([P, NDC, FKV], f32r)
            wo_sb = wpool.tile([P, NQC, D], f32r)
            nc.sync.dma_start(wq_sb[:], wq_d.rearrange("(ko p) f -> p ko f", p=P))
            nc.sync.dma_start(wk_sb[:], wk_d.rearrange("(ko p) f -> p ko f", p=P))
            nc.sync.dma_start(wv_sb[:], wv_d.rearrange("(ko p) f -> p ko f", p=P))
            nc.sync.dma_start(wo_sb[:], wo_d.rearrange("(ko p) f -> p ko f", p=P))

            rqT_sb = cpool.tile([P, P], f32r)
            rkT_sb = cpool.tile([P, P], f32r)
            hsel_sb = cpool.tile([P, 2], f32r)
            hexp_sb = cpool.tile([2, P], f32r)
            e1_sb = cpool.tile([1, P], f32r)
            masks_sb = cpool.tile([P, 4, TB], f32r)
            ident_sb = cpool.tile([P, P], f32r)
            nc.sync.dma_start(rqT_sb[:], rqT_d[:])
            nc.sync.dma_start(rkT_sb[:], rkT_d[:])
            nc.sync.dma_start(hsel_sb[:], hsel_d[:])
            nc.sync.dma_start(hexp_sb[:], hexp_d[:])
            nc.sync.dma_start(e1_sb[:], e1_d[:])
            nc.sync.dma_start(masks_sb[:], masks_d[:])
            nc.sync.dma_start(ident_sb[:], ident_d[:])

            # K^T (per-kv-head at both partition halves) and V (+ones col)
            ktf = kvpool.tile([P, T], f32r)          # rows 0:64 kv0, 64:128 kv1
            kts = kvpool.tile([P, T], f32r)          # swapped halves
            v_sb = kvpool.tile([P, NKC, KVL, 66], f32r)  # [tok, kc, g, hd+ones+pad]
            ones_bc = nc.const_aps.tensor(1.0, (P, NKC, KVL, 66), f32)
            nc.vector.tensor_copy(v_sb[:], ones_bc)

            for tb in range(NTB):
                tbs = slice(tb * TB, (tb + 1) * TB)

                cq_t = trig.tile([P, TB], f32, tag="cq")
                ck_t = trig.tile([P, TB], f32, tag="ck")
                sn_t = trig.tile([P, TB], f32, tag="sn")
                nc.sync.dma_start(cq_t[:], cosq_d[:, tbs])
                nc.sync.dma_start(ck_t[:], cosk_d[:, tbs])
                nc.sync.dma_start(sn_t[:], sin_d[:, tbs])

                # ---- A: projections ----
                qps = [psum.tile([P, TB], f32, tag="big", name=f"qps{_f}") for _f in range(NQC)]
                kps = psum.tile([P, TB], f32, tag="big")
                vps = psum.tile([P, TB], f32, tag="big")
                for dc in range(NDC):
                    xt = xpool.tile([P, TB], f32r)
                    nc.sync.dma_start(xt[:], xT_d[dc * P:(dc + 1) * P, tbs])
                    st = dc == 0
                    sp = dc == NDC - 1
                    for fc in range(NQC):
                        nc.tensor.matmul(qps[fc][:], wq_sb[:, dc, fc * P:(fc + 1) * P],
                                         xt[:], start=st, stop=sp)
                    nc.tensor.matmul(kps[:], wk_sb[:, dc, :], xt[:], start=st, stop=sp)
                    nc.tensor.matmul(vps[:], wv_sb[:, dc, :], xt[:], start=st, stop=sp)

                # ---- B: RMSNorm + RoPE on Q chunks and K ----
                qts = []
                for ci in range(NQC + 1):
                    is_k = ci == NQC
                    cps = kps if is_k else qps[ci]
                    rT = rkT_sb if is_k else rqT_sb
                    ct = ck_t if is_k else cq_t

                    qsb = btmp.tile([P, TB], f32r, tag="qsb")
                    nc.vector.tensor_copy(qsb[:], cps[:])
                    sq = btmp.tile([P, TB], f32r, tag="sq")
                    nc.scalar.square(sq[:], cps[:])
                    ss = psmall.tile([2, TB], f32, tag="sps", name="ss")
                    nc.tensor.matmul(ss[:], hsel_sb[:], sq[:], start=True, stop=True)
                    lnb = spool.tile([2, TB], f32, tag="lnb")
                    nc.scalar.activation(lnb[:], ss[:], AF.Ln, bias=EPS, scale=1.0 / HD)
                    rr = spool.tile([2, TB], f32r, tag="rr")
                    nc.scalar.activation(rr[:], lnb[:], AF.Exp, scale=-0.5)
                    bc = psum.tile([P, TB], f32, tag="big")
                    nc.tensor.matmul(bc[:], hexp_sb[:], rr[:], start=True, stop=True)
                    rot = psum.tile([P, TB], f32, tag="big")
                    nc.tensor.matmul(rot[:], rT[:], qsb[:], start=True, stop=True)
                    m1 = btmp.tile([P, TB], f32, tag="m1")
                    nc.vector.tensor_tensor(m1[:], qsb[:], ct[:], ALU.mult)
                    m2 = btmp.tile([P, TB], f32, tag="m2")
                    nc.vector.tensor_tensor(m2[:], rot[:], sn_t[:], ALU.mult)
                    s12 = btmp.tile([P, TB], f32, tag="m1", name="s12")
                    nc.vector.tensor_tensor(s12[:], m1[:], m2[:], ALU.add)
                    if not is_k:
                        qt = qpool.tile([P, TB], f32r, tag=f"qt{ci}")
                        nc.vector.tensor_tensor(qt[:], s12[:], bc[:], ALU.mult)
                        qts.append(qt)
                    else:
                        nc.vector.tensor_tensor(ktf[:, tbs], s12[:], bc[:], ALU.mult)
                        nc.vector.tensor_tensor(kts[0:64, tbs], s12[64:P], bc[64:P], ALU.mult)
                        nc.vector.tensor_tensor(kts[64:P, tbs], s12[0:64], bc[0:64], ALU.mult)

                # ---- C: V transpose into [tok, hd] with ones column ----
                vt_sb = btmp.tile([P, TB], f32r, tag="sq", name="vt_sb")
                nc.vector.tensor_copy(vt_sb[:], vps[:])
                for st4 in range(TB // P):
                    kc = tb * (TB // P) + st4
                    tp = psum.tile([P, P], f32r, tag="big")
                    nc.tensor.transpose(tp[:], vt_sb[:, st4 * P:(st4 + 1) * P], ident_sb[:])
                    nc.vector.tensor_copy(v_sb[:, kc, 0, 0:64], tp[:, 0:64])
                    nc.vector.tensor_copy(v_sb[:, kc, 1, 0:64], tp[:, 64:P])

                # ---- D: attention for query block tb ----
                nkc = (tb + 1) * (TB // P)
                for g in range(KVL):
                    o_ps = [psum.tile([65, TB], f32, tag="big", name=f"ops{_h}") for _h in range(GROUPS)]
                    for kc in range(nkc):
                        kslice = slice(kc * P, (kc + 1) * P)
                        for hj in range(GROUPS):
                            hl = GROUPS * g + hj
                            bq = 64 * (hl % 2)
                            cf = hl // 2
                            kt_tile = ktf if bq == 64 * g else kts
                            sps = psmall.tile([P, TB], f32, tag="sps", name="sps")
                            nc.tensor.matmul(sps[:], kt_tile[bq:bq + 64, kslice],
                                             qts[cf][bq:bq + 64, :], start=True, stop=True)
                            es = epool.tile([P, TB], f32r, tag="es")
                            nc.scalar.activation(es[:], sps[:], AF.Exp, scale=float(SCALE))
                            tdiag = kc - tb * (TB // P)
                            if tdiag >= 0:
                                nc.vector.tensor_tensor(es[:], es[:], masks_sb[:, tdiag, :], ALU.mult)
                            nc.tensor.matmul(o_ps[hj][:], v_sb[:, kc, g, 0:65], es[:],
                                             start=(kc == 0), stop=(kc == nkc - 1))
                    # normalize + pack head pairs for the O projection
                    for pj in range(2):
                        cf = 2 * g + pj
                        hA = 2 * pj
                        hB = hA + 1
                        ldA = spool.tile([1, TB], f32, tag="ld", name="ldA")
                        nc.scalar.activation(ldA[:], o_ps[hA][64:65, :], AF.Ln)
                        ldB = spool.tile([1, TB], f32, tag="ld", name="ldB")
                        nc.scalar.activation(ldB[:], o_ps[hB][64:65, :], AF.Ln)
                        rpA = spool.tile([1, TB], f32r, tag="rp", name="rpA")
                        nc.scalar.activation(rpA[:], ldA[:], AF.Exp, scale=-1.0)
                        rpB = spool.tile([1, TB], f32r, tag="rp", name="rpB")
                        nc.scalar.activation(rpB[:], ldB[:], AF.Exp, scale=-1.0)
                        bc2 = psum.tile([P, TB], f32, tag="big")
                        nc.tensor.matmul(bc2[:], hexp_sb[0:1, :], rpA[:], start=True, stop=False)
                        nc.tensor.matmul(bc2[:], e1_sb[:], rpB[:], start=False, stop=True)
                        osb = opool.tile([P, TB], f32, tag="osb")
                        nc.vector.tensor_copy(osb[0:64, :], o_ps[hA][0:64, :])
                        nc.vector.tensor_copy(osb[64:P, :], o_ps[hB][0:64, :])
                        orhs = opool.tile([P, TB], f32r, tag=f"orhs{cf}")
                        nc.vector.tensor_tensor(orhs[:], osb[:], bc2[:], ALU.mult)
                        if g == 0 and pj == 0:
                            orhs_list = [None] * NQC
                        orhs_list[cf] = orhs

                # ---- E: output projection for this token block ----
                for dc2 in range(NDC):
                    ops_ = psum.tile([P, TB], f32, tag="big")
                    for cf in range(NQC):
                        nc.tensor.matmul(ops_[:], wo_sb[:, cf, dc2 * P:(dc2 + 1) * P],
                                         orhs_list[cf][:], start=(cf == 0), stop=(cf == NQC - 1))
                    ob = outp.tile([P, TB], f32, tag="ob")
                    nc.vector.tensor_copy(ob[:], ops_[:])
                    nc.sync.dma_start(outT_d[dc2 * P:(dc2 + 1) * P, tbs], ob[:])

    nc.compile()
    return nc


_NC_CACHE = None


def _get_nc():
    global _NC_CACHE
    if _NC_CACHE is None:
        _NC_CACHE = _build_nc()
    return _NC_CACHE


def _host_constants(q_scale, k_scale):
    pos = np.arange(T, dtype=np.float64)
    invf = 1.0 / (THETA ** (np.arange(0, HD, 2, dtype=np.float64) / HD))  # (32,)
    ang = pos[:, None] * invf[None, :]                                    # (T, 32)
    c = np.cos(ang)
    s = np.sin(ang)
    pidx = np.arange(P) % 32
    hidx = np.arange(P) % HD
    cosq = (c[:, pidx].T * q_scale[hidx][:, None]).astype(np.float32)     # (128, T)
    cosk = (c[:, pidx].T * k_scale[hidx][:, None]).astype(np.float32)
    sin = s[:, pidx].T.astype(np.float32)

    def rmat(scale):
        R = np.zeros((HD, HD), dtype=np.float64)
        for i in range(32):
            R[i, i + 32] = -scale[i + 32]
            R[i + 32, i] = scale[i]
        M = np.kron(np.eye(2), R)
        return np.ascontiguousarray(M.T.astype(np.float32))

    hsel = np.zeros((P, 2), dtype=np.float32)
    hsel[0:64, 0] = 1.0
    hsel[64:P, 1] = 1.0
    hexp = np.ascontiguousarray(hsel.T)

    masks = np.zeros((P, 4, TB), dtype=np.float32)
    pp = np.arange(P)[:, None]
    ff = np.arange(TB)[None, :]
    for t in range(4):
        masks[:, t, :] = (ff >= pp + P * t).astype(np.float32)

    ident = np.eye(P, dtype=np.float32)
    return cosq, cosk, sin, rmat(q_scale), rmat(k_scale), hsel, hexp, masks, ident


def _run(inputs, trace=False):
    x = np.asarray(inputs["x"], dtype=np.float32)
    Wq = np.asarray(inputs["Wq"], dtype=np.float32)
    Wk = np.asarray(inputs["Wk"], dtype=np.float32)
    Wv = np.asarray(inputs["Wv"], dtype=np.float32)
    Wo = np.asarray(inputs["Wo"], dtype=np.float32)
    q_scale = np.asarray(inputs["q_scale"], dtype=np.float64)
    k_scale = np.asarray(inputs["k_scale"], dtype=np.float64)

    cosq, cosk, sin, rqT, rkT, hsel, hexp, masks, ident = _host_constants(q_scale, k_scale)

    in_maps = []
    for c in range(8):
        b = c // 4
        r = c % 4
        in_maps.append({
            "xT": np.ascontiguousarray(x[b].T),
            "wq": np.ascontiguousarray(Wq[:, r * FQ:(r + 1) * FQ]),
            "wk": np.ascontiguousarray(Wk[:, r * FKV:(r + 1) * FKV]),
            "wv": np.ascontiguousarray(Wv[:, r * FKV:(r + 1) * FKV]),
            "wo": np.ascontiguousarray(Wo[r * FQ:(r + 1) * FQ, :]),
            "cosq": cosq, "cosk": cosk, "sin": sin,
            "rqT": rqT, "rkT": rkT, "hsel": hsel, "hexp": hexp,
            "e1": np.ascontiguousarray(hexp[1:2, :]),
            "masks": masks, "ident": ident,
        })

    nc = _get_nc()
    res = run_bass_kernel_spmd(nc, in_maps, core_ids=list(range(8)), trace=trace)
    out = np.empty((B, T, D), dtype=np.float32)
    for b in range(B):
        acc = res.results[4 * b]["outT"].astype(np.float32).copy()
        for r in range(1, 4):
            acc += res.results[4 * b + r]["outT"]
        out[b] = acc.T
    return out, res


def kernel(**inputs):
    out, _ = _run(inputs, trace=False)
    return out



# revision 10
# speedup vs baseline: 1.5640x; 1.5640x over previous
"""GQA (32 q heads / 8 kv heads, head_dim 64, causal, QK-RMSNorm + RoPE) on 8 TRN2 cores.

Sharding: data-parallel over batch (2) x tensor-parallel over heads (4):
each core handles one batch element, 8 query heads, 2 kv heads, and produces
a partial output (its heads' slice of the Wo contraction); the host sums the
4 partials per batch element.

v2: all matmuls in bf16 (fp32r triggered HAM activity throttling to 50% duty),
grouped-head attention matmuls (heads of a kv-group share the K/V stationary
operand -> fewer matmul+LDWEIGHTS), batched exp over 4 heads with 256-query
granularity for finer causal trimming, softmax 1/denom on DVE (reciprocal)
instead of Ln/Exp on ACT (kills activation-table ping-pong), batched Ln/Exp for
the RMSNorm rsqrt, V transposed via DMA XBAR instead of the PE.
"""

import numpy as np
import ml_dtypes

import concourse.bass as bass
import concourse.mybir as mybir
import concourse.tile as tile
from concourse import bacc
from concourse.bass_utils import run_bass_kernel_spmd

# Problem config (hardcoded per contract)
B, T, D = 2, 2048, 2048
H, KV, HD = 32, 8, 64
GROUPS = H // KV
THETA = 10000.0
SCALE = 1.0 / np.sqrt(HD)
EPS = 1e-6

# Per-core sharding
HQL = H // 4          # 8 local q heads
KVL = KV // 4         # 2 local kv heads (= groups per core)
FQ = HQL * HD         # 512
FKV = KVL * HD        # 128

# Tiling
P = 128
TB = 512              # token block (phase 1)
TQ = 256              # query sub-block (phase 2)
NTB = T // TB         # 4
NDC = D // P          # 16 contraction chunks
NKC = T // P          # 16 key chunks
NQC = FQ // P         # 4 q-proj chunks (2 heads each)

f32 = mybir.dt.float32
bf16 = mybir.dt.bfloat16
AF = mybir.ActivationFunctionType
ALU = mybir.AluOpType


def _build_nc():
    nc = bacc.Bacc("TRN2", target_bir_lowering=False, debug=False, num_devices=8)

    eps_t = nc.alloc_sbuf_tensor("const-f32-eps", [128, 1], f32)
    nc.gpsimd.memset(eps_t.ap(), EPS)
    nc.const_aps.aps[(f32, EPS)] = eps_t.ap()
    nc.all_engine_barrier()

    xT_d = nc.dram_tensor("xT", [D, T], bf16, kind="ExternalInput")
    wq_d = nc.dram_tensor("wq", [D, FQ], bf16, kind="ExternalInput")
    wk_d = nc.dram_tensor("wk", [D, FKV], bf16, kind="ExternalInput")
    wv_d = nc.dram_tensor("wv", [D, FKV], bf16, kind="ExternalInput")
    wo_d = nc.dram_tensor("wo", [P, NQC, D], bf16, kind="ExternalInput")
    cosq_d = nc.dram_tensor("cosq", [P, T], f32, kind="ExternalInput")
    cosk_d = nc.dram_tensor("cosk", [P, T], f32, kind="ExternalInput")
    sin_d = nc.dram_tensor("sin", [P, T], f32, kind="ExternalInput")
    rqT_d = nc.dram_tensor("rqT", [P, P], bf16, kind="ExternalInput")
    rkT_d = nc.dram_tensor("rkT", [P, P], bf16, kind="ExternalInput")
    hsel_d = nc.dram_tensor("hsel", [P, 2], bf16, kind="ExternalInput")
    hexp_d = nc.dram_tensor("hexp", [2, P], bf16, kind="ExternalInput")
    one64_d = nc.dram_tensor("one64", [1, 64], bf16, kind="ExternalInput")
    masks_d = nc.dram_tensor("masks", [P, 2, GROUPS, TQ], bf16, kind="ExternalInput")
    ident_d = nc.dram_tensor("ident", [P, P], bf16, kind="ExternalInput")
    outT_d = nc.dram_tensor("outT", [D, T], f32, kind="ExternalOutput")

    with tile.TileContext(nc) as tc:
        with (
            tc.tile_pool(name="wpool", bufs=1) as wpool,
            tc.tile_pool(name="cpool", bufs=1) as cpool,
            tc.tile_pool(name="kvpool", bufs=1) as kvpool,
            tc.tile_pool(name="xpool", bufs=2) as xpool,
            tc.tile_pool(name="trig", bufs=2) as trig,
            tc.tile_pool(name="bpool", bufs=2) as bpool,
            tc.tile_pool(name="epool", bufs=3) as epool,
            tc.tile_pool(name="outp", bufs=2) as outp,
        ):
            # ---- persistent weights / constants ----
            wq_sb = wpool.tile([P, NDC, FQ], bf16)
            wk_sb = wpool.tile([P, NDC, FKV], bf16)
            wv_sb = wpool.tile([P, NDC, FKV], bf16)
            wo_sb = wpool.tile([P, NQC, D], bf16)
            nc.sync.dma_start(wq_sb[:], wq_d.rearrange("(ko p) f -> p ko f", p=P))
            nc.sync.dma_start(wk_sb[:], wk_d.rearrange("(ko p) f -> p ko f", p=P))
            nc.sync.dma_start(wv_sb[:], wv_d.rearrange("(ko p) f -> p ko f", p=P))
            nc.sync.dma_start(wo_sb[:], wo_d[:])

            rqT_sb = cpool.tile([P, P], bf16)
            rkT_sb = cpool.tile([P, P], bf16)
            hsel_sb = cpool.tile([P, 2], bf16)
            hexp_sb = cpool.tile([2, P], bf16)
            one64_sb = cpool.tile([1, 64], bf16)
            masks_sb = cpool.tile([P, 2, GROUPS, TQ], bf16)
            ident_sb = cpool.tile([P, P], bf16)
            nc.sync.dma_start(ident_sb[:], ident_d[:])
            nc.sync.dma_start(rqT_sb[:], rqT_d[:])
            nc.sync.dma_start(rkT_sb[:], rkT_d[:])
            nc.sync.dma_start(hsel_sb[:], hsel_d[:])
            nc.sync.dma_start(hexp_sb[:], hexp_d[:])
            nc.sync.dma_start(one64_sb[:], one64_d[:])
            nc.sync.dma_start(masks_sb[:], masks_d[:])

            # K^T per group on partition halves; V [key, kc, g, hd+ones];
            # Q packed [64g+hd partitions, head-in-group slot, token]
            ktf = kvpool.tile([P, T], bf16)
            v_sb = kvpool.tile([P, NKC, KVL, 66], bf16)
            qg = kvpool.tile([P, GROUPS, T], bf16)
            ones_bc = nc.const_aps.tensor(1.0, (P, NKC, KVL, 66), f32)
            nc.vector.tensor_copy(v_sb[:], ones_bc)

            # ---------------- Phase 1: projections + QK norm/rope ----------
            with (
                tc.tile_pool(name="psA", bufs=6, space="PSUM") as psA,
                tc.tile_pool(name="psB", bufs=2, space="PSUM") as psB,
            ):
                for tb in range(NTB):
                    tbs = slice(tb * TB, (tb + 1) * TB)

                    cq_t = trig.tile([P, TB], f32, tag="cq")
                    ck_t = trig.tile([P, TB], f32, tag="ck")
                    sn_t = trig.tile([P, TB], f32, tag="sn")
                    nc.sync.dma_start(cq_t[:], cosq_d[:, tbs])
                    nc.sync.dma_start(ck_t[:], cosk_d[:, tbs])
                    nc.sync.dma_start(sn_t[:], sin_d[:, tbs])

                    xtb = xpool.tile([P, NDC, TB], bf16, tag="x")
                    for dc in range(NDC):
                        nc.sync.dma_start(xtb[:, dc, :],
                                          xT_d[dc * P:(dc + 1) * P, tbs])

                    # projections: V, K, then 4 Q chunks
                    vps = psA.tile([P, TB], f32, tag="big", name="vps")
                    for dc in range(NDC):
                        nc.tensor.matmul(vps[:], wv_sb[:, dc, :], xtb[:, dc, :],
                                         start=dc == 0, stop=dc == NDC - 1)
                    vt = bpool.tile([P, TB], bf16, tag="vt")
                    nc.scalar.copy(vt[:], vps[:])
                    # V transpose on the PE (bf16), both groups per 128-chunk
                    for st4 in range(TB // P):
                        kc = tb * (TB // P) + st4
                        tp = psB.tile([P, P], bf16, tag="small", name="tp")
                        nc.tensor.transpose(tp[:], vt[:, st4 * P:(st4 + 1) * P],
                                            ident_sb[:])
                        nc.vector.tensor_copy(v_sb[:, kc, 0, 0:64], tp[:, 0:64])
                        nc.vector.tensor_copy(v_sb[:, kc, 1, 0:64], tp[:, 64:P])

                    kps = psA.tile([P, TB], f32, tag="big", name="kps")
                    for dc in range(NDC):
                        nc.tensor.matmul(kps[:], wk_sb[:, dc, :], xtb[:, dc, :],
                                         start=dc == 0, stop=dc == NDC - 1)

                    qps = [psA.tile([P, TB], f32, tag="big", name=f"qps{c}")
                           for c in range(NQC)]
                    for dc in range(NDC):
                        for c in range(NQC):
                            nc.tensor.matmul(qps[c][:],
                                             wq_sb[:, dc, c * P:(c + 1) * P],
                                             xtb[:, dc, :],
                                             start=dc == 0, stop=dc == NDC - 1)

                    # pass 1: bf16 copies + per-token sum of squares
                    chunks = qps + [kps]
                    qsb = []
                    ss_sb = bpool.tile([2, NQC + 1, TB], f32, tag="ss_sb", bufs=1)
                    for ci, cps in enumerate(chunks):
                        qs_t = bpool.tile([P, TB], bf16, tag=f"qsb{ci}")
                        nc.scalar.copy(qs_t[:], cps[:])
                        qsb.append(qs_t)
                        sq = bpool.tile([P, TB], bf16, tag="sq")
                        nc.vector.tensor_tensor(sq[:], qs_t[:], qs_t[:], ALU.mult)
                        ssp = psB.tile([2, TB], f32, tag="small", name="ssp")
                        nc.tensor.matmul(ssp[:], hsel_sb[:], sq[:],
                                         start=True, stop=True)
                        nc.scalar.copy(ss_sb[:, ci, :], ssp[:])

                    # batched rsqrt: rr = exp(-0.5 * ln(ss/HD + eps))
                    lnb = bpool.tile([2, NQC + 1, TB], f32, tag="lnb", bufs=1)
                    nc.scalar.activation(lnb[:], ss_sb[:], AF.Ln,
                                         bias=EPS, scale=1.0 / HD)
                    rr = bpool.tile([2, NQC + 1, TB], bf16, tag="rr", bufs=1)
                    nc.scalar.activation(rr[:], lnb[:], AF.Exp, scale=-0.5)

                    # pass 2: rope + apply rsqrt, write qg / ktf
                    for ci in range(NQC + 1):
                        is_k = ci == NQC
                        rT = rkT_sb if is_k else rqT_sb
                        ct = ck_t if is_k else cq_t
                        bc = psB.tile([P, TB], f32, tag="small", name="bc")
                        nc.tensor.matmul(bc[:], hexp_sb[:], rr[:, ci, :],
                                         start=True, stop=True)
                        rot = psB.tile([P, TB], f32, tag="small", name="rot")
                        nc.tensor.matmul(rot[:], rT[:], qsb[ci][:],
                                         start=True, stop=True)
                        m1 = bpool.tile([P, TB], bf16, tag="m1")
                        nc.vector.tensor_tensor(m1[:], qsb[ci][:], ct[:], ALU.mult)
                        m2 = bpool.tile([P, TB], bf16, tag="m2")
                        nc.vector.tensor_tensor(m2[:], rot[:], sn_t[:], ALU.mult)
                        s12 = bpool.tile([P, TB], bf16, tag="s12")
                        nc.vector.tensor_tensor(s12[:], m1[:], m2[:], ALU.add)
                        if not is_k:
                            g = ci // 2
                            j0 = 2 * (ci % 2)
                            gp = slice(64 * g, 64 * g + 64)
                            nc.vector.tensor_tensor(qg[gp, j0, tbs],
                                                    s12[0:64], bc[0:64], ALU.mult)
                            nc.vector.tensor_tensor(qg[gp, j0 + 1, tbs],
                                                    s12[64:P], bc[64:P], ALU.mult)
                        else:
                            nc.vector.tensor_tensor(ktf[0:64, tbs],
                                                    s12[0:64], bc[0:64], ALU.mult)
                            nc.vector.tensor_tensor(ktf[64:P, tbs],
                                                    s12[64:P], bc[64:P], ALU.mult)

            # ---------------- Phase 2: attention + output projection --------
            with (
                tc.tile_pool(name="psSP", bufs=2, space="PSUM") as psSP,
                tc.tile_pool(name="psO", bufs=2, space="PSUM") as psO,
                tc.tile_pool(name="psM", bufs=2, space="PSUM") as psM,
            ):
                for tb in range(NTB):
                    orhs = bpool.tile([P, GROUPS, TB], bf16, tag="orhs")
                    for g in range(KVL):
                        gp = slice(64 * g, 64 * g + 64)
                        for qh in range(2):
                            qbase = tb * TB + qh * TQ
                            qs = slice(qbase, qbase + TQ)
                            qsl = slice(qh * TQ, (qh + 1) * TQ)
                            nkc = qbase // P + 2
                            o01 = psO.tile([65, 2, TQ], f32, tag="o", name="o01")
                            o23 = psO.tile([65, 2, TQ], f32, tag="o", name="o23")
                            es_l = [None] * nkc
                            # software pipeline: AV(kc) trails exp(kc) by one step
                            for kc in range(nkc + 1):
                                if kc < nkc:
                                    sps = psSP.tile([P, GROUPS, TQ], f32, tag="sps")
                                    nc.tensor.matmul(sps[:, 0:2, :], ktf[gp, kc * P:(kc + 1) * P],
                                                     qg[gp, 0:2, qs],
                                                     start=True, stop=True)
                                    nc.tensor.matmul(sps[:, 2:4, :], ktf[gp, kc * P:(kc + 1) * P],
                                                     qg[gp, 2:4, qs],
                                                     start=True, stop=True)
                                    es = epool.tile([P, GROUPS, TQ], bf16, tag="es")
                                    nc.scalar.activation(es[:], sps[:], AF.Exp,
                                                         scale=float(SCALE))
                                    mval = kc * P - qbase
                                    if mval >= 0:
                                        nc.vector.tensor_tensor(
                                            es[:], es[:],
                                            masks_sb[:, mval // P, :, :], ALU.mult)
                                    es_l[kc] = es
                                if kc >= 1:
                                    kp = kc - 1
                                    nc.tensor.matmul(o01[:], v_sb[:, kp, g, 0:65],
                                                     es_l[kp][:, 0:2, :],
                                                     start=kp == 0, stop=kp == nkc - 1)
                                    nc.tensor.matmul(o23[:], v_sb[:, kp, g, 0:65],
                                                     es_l[kp][:, 2:4, :],
                                                     start=kp == 0, stop=kp == nkc - 1)
                            # normalize: orhs = o / denom  (denom in row 64)
                            den01 = bpool.tile([1, 2, TQ], bf16, tag="den01")
                            den23 = bpool.tile([1, 2, TQ], bf16, tag="den23")
                            with nc.allow_low_precision(reason="softmax 1/denom in bf16 is within tolerance"):
                                nc.vector.reciprocal(den01[:], o01[64:65, :, :])
                                nc.vector.reciprocal(den23[:], o23[64:65, :, :])
                            bc201 = bpool.tile([64, 2, TQ], bf16, tag="bc201")
                            nc.gpsimd.partition_broadcast(bc201[:], den01[:])
                            bc223 = bpool.tile([64, 2, TQ], bf16, tag="bc223")
                            nc.gpsimd.partition_broadcast(bc223[:], den23[:])
                            nc.vector.tensor_tensor(orhs[gp, 0:2, qsl],
                                                    o01[0:64, :, :], bc201[:],
                                                    ALU.mult)
                            nc.vector.tensor_tensor(orhs[gp, 2:4, qsl],
                                                    o23[0:64, :, :], bc223[:],
                                                    ALU.mult)

                    # output projection for this token block
                    tbs = slice(tb * TB, (tb + 1) * TB)
                    for dc2 in range(NDC):
                        acc = psM.tile([P, TB], f32, tag="m", name="acc")
                        for j in range(GROUPS):
                            nc.tensor.matmul(acc[:],
                                             wo_sb[:, j, dc2 * P:(dc2 + 1) * P],
                                             orhs[:, j, :],
                                             start=j == 0, stop=j == GROUPS - 1)
                        ob = outp.tile([P, TB], f32, tag="ob")
                        nc.scalar.copy(ob[:], acc[:])
                        nc.sync.dma_start(outT_d[dc2 * P:(dc2 + 1) * P, tbs], ob[:])

    nc.compile()
    return nc


_NC_CACHE = None


def _get_nc():
    global _NC_CACHE
    if _NC_CACHE is None:
        _NC_CACHE = _build_nc()
    return _NC_CACHE


def _host_constants(q_scale, k_scale):
    pos = np.arange(T, dtype=np.float64)
    invf = 1.0 / (THETA ** (np.arange(0, HD, 2, dtype=np.float64) / HD))  # (32,)
    ang = pos[:, None] * invf[None, :]                                    # (T, 32)
    c = np.cos(ang)
    s = np.sin(ang)
    pidx = np.arange(P) % 32
    hidx = np.arange(P) % HD
    cosq = (c[:, pidx].T * q_scale[hidx][:, None]).astype(np.float32)     # (128, T)
    cosk = (c[:, pidx].T * k_scale[hidx][:, None]).astype(np.float32)
    sin = s[:, pidx].T.astype(np.float32)

    def rmat(scale):
        R = np.zeros((HD, HD), dtype=np.float64)
        for i in range(32):
            R[i, i + 32] = -scale[i + 32]
            R[i + 32, i] = scale[i]
        M = np.kron(np.eye(2), R)
        return np.ascontiguousarray(M.T).astype(ml_dtypes.bfloat16)

    hsel = np.zeros((P, 2), dtype=np.float32)
    hsel[0:64, 0] = 1.0
    hsel[64:P, 1] = 1.0
    hexp = np.ascontiguousarray(hsel.T).astype(ml_dtypes.bfloat16)
    hsel = hsel.astype(ml_dtypes.bfloat16)
    one64 = np.ones((1, 64), dtype=ml_dtypes.bfloat16)

    # masks[p, i, j, f] = (f >= p + 128*i), replicated over the 4 head slots
    pp = np.arange(P)[:, None]
    ff = np.arange(TQ)[None, :]
    masks = np.zeros((P, 2, GROUPS, TQ), dtype=np.float32)
    for i in range(2):
        m = (ff >= pp + P * i).astype(np.float32)
        for j in range(GROUPS):
            masks[:, i, j, :] = m
    masks = masks.astype(ml_dtypes.bfloat16)
    ident = np.eye(P, dtype=ml_dtypes.bfloat16)

    return cosq, cosk, sin, rmat(q_scale), rmat(k_scale), hsel, hexp, one64, masks, ident


def _run(inputs, trace=False):
    x = np.asarray(inputs["x"], dtype=np.float32)
    Wq = np.asarray(inputs["Wq"], dtype=np.float32)
    Wk = np.asarray(inputs["Wk"], dtype=np.float32)
    Wv = np.asarray(inputs["Wv"], dtype=np.float32)
    Wo = np.asarray(inputs["Wo"], dtype=np.float32)
    q_scale = np.asarray(inputs["q_scale"], dtype=np.float64)
    k_scale = np.asarray(inputs["k_scale"], dtype=np.float64)

    cosq, cosk, sin, rqT, rkT, hsel, hexp, one64, masks, ident = _host_constants(
        q_scale, k_scale)

    in_maps = []
    for cid in range(8):
        b = cid // 4
        r = cid % 4
        # Wo rows (g, head j within group, hd) -> [64g+hd partitions, j slots]
        wo_loc = Wo[r * FQ:(r + 1) * FQ, :].reshape(KVL, GROUPS, HD, D)
        wo_loc = np.ascontiguousarray(
            wo_loc.transpose(0, 2, 1, 3)).reshape(P, GROUPS, D)
        in_maps.append({
            "xT": np.ascontiguousarray(x[b].T).astype(ml_dtypes.bfloat16),
            "wq": np.ascontiguousarray(
                Wq[:, r * FQ:(r + 1) * FQ]).astype(ml_dtypes.bfloat16),
            "wk": np.ascontiguousarray(
                Wk[:, r * FKV:(r + 1) * FKV]).astype(ml_dtypes.bfloat16),
            "wv": np.ascontiguousarray(
                Wv[:, r * FKV:(r + 1) * FKV]).astype(ml_dtypes.bfloat16),
            "wo": wo_loc.astype(ml_dtypes.bfloat16),
            "cosq": cosq, "cosk": cosk, "sin": sin,
            "rqT": rqT, "rkT": rkT, "hsel": hsel, "hexp": hexp,
            "one64": one64, "masks": masks, "ident": ident,
        })

    nc = _get_nc()
    res = run_bass_kernel_spmd(nc, in_maps, core_ids=list(range(8)), trace=trace)
    out = np.empty((B, T, D), dtype=np.float32)
    for b in range(B):
        acc = res.results[4 * b]["outT"].astype(np.float32).copy()
        for r in range(1, 4):
            acc += res.results[4 * b + r]["outT"]
        out[b] = acc.T
    return out, res


def kernel(**inputs):
    out, _ = _run(inputs, trace=False)
    return out
